# revision 21
# baseline (speedup 1.0000x reference)
"""Trainium2 Bass kernel for nn_BinarizeLayer.

out[b, f] = (medians[f] > 0) AND (inputs[b, f] >= medians[f])

Host preprocessing folds the two conditions into one comparison:
m2[f] = medians[f] if medians[f] > 0 else +inf, so out = inputs >= m2
(inputs are finite, so x >= +inf is always False).

Data-parallel over 8 NeuronCores: each core handles a 2048-row slice of
the 16384x8192 f32 input, processed as 64 chunks of 32 consecutive rows
(1 MiB, fully contiguous in DRAM). The load's access pattern fans the
chunk onto 128 partitions: partition p holds quarter-row
(row 32i + p//4, cols (p%4)*2048 ..), so HBM reads stay sequential
while compute still uses all 128 lanes. ALL loads ride the SP HWDGE
ring as ONE sequential address stream — splitting them across two
descriptor queues (adjacent- or far-interleaved, HWDGE or SWDGE)
measured 20-30% slower, because the SDMA engines round-robin packets
between queues and HBM loses stream locality. The ACT ring carries
only the constants, PSUM evacuations and output stores, so no load
dispatch is ever queued behind compute.

Per chunk the DVE compares against a median tile host-prepared in the
same per-partition layout, emitting 0/1 bf16 bits. Chunks are paired:
the even chunk's four [128,16]x[128,512] pack matmuls land at PSUM
partition base 0, the odd chunk's at base 32 (PE output bases are
restricted to 0/32/64), so ONE ACT copy of [64, 2048] evacuates both
chunks' packed bits in the cycles a per-chunk [16, 2048] copy would
spend on one (ACT cycles scale with free-dim only). Packed bytes
accumulate in SBUF and leave as a handful of large plain-AP stores
(one per 16-partition half; composite partition APs on a store's SBUF
side scramble partitions); each core stores 2 MiB instead of 16 MiB.
The first chunk is loaded and compared in 512-column pieces so the DVE
starts early; the last chunk likewise, with its PSUM evacuation
tapered per col-block, so the post-load drain is one small piece
chain. The host unpacks bits and inverts the partition bijection with
pure reshapes.
"""

import os
import time

os.environ.setdefault("JAX_PLATFORMS", "cpu,axon")

import numpy as np

import concourse.bacc as bacc
import concourse.mybir as mybir
from concourse import tile
from concourse.bass_utils import run_bass_kernel_spmd

N_CORES = 8
B, F = 16384, 8192
BS = B // N_CORES  # rows per core
P = 128  # SBUF partitions
R = 32  # rows per chunk
CQ = F // (P // R)  # columns per partition quarter-row (2048)
N_CHUNKS = BS // R  # chunks per core (64)
G = P // 8  # pack matmul output rows (16)
MM_N = 512  # pack matmul free-dim chunk (one PSUM bank)
N_SUPER = N_CHUNKS // 2  # chunk pairs (32)
# Output store groups (in superchunks): big stores early, tiny last
ST_GROUPS = [(0, 8), (8, 16), (16, 24), (24, 31), (31, 32)]


def _build():
    nc = bacc.Bacc(
        "TRN2",
        target_bir_lowering=False,
        debug=False,
        num_devices=N_CORES,
    )
    x = nc.declare_dram_parameter("x", [BS, F], mybir.dt.float32, isOutput=False)
    med = nc.declare_dram_parameter("med", [P, CQ], mybir.dt.float32, isOutput=False)
    pw = nc.declare_dram_parameter("pw", [P, G], mybir.dt.float32, isOutput=False)
    # out[h, g, s, j]: bits of x-partitions [8g, 8g+8) of chunk 2s+h,
    # chunk-col j.
    out = nc.declare_dram_parameter(
        "out", [2, G, N_SUPER, CQ], mybir.dt.uint8, isOutput=True
    )
    # Chunk view: x as [N_CHUNKS, 128 partitions, 2048], partition
    # p = (row-in-chunk p//4, quarter p%4); DRAM order stays row-major,
    # so each chunk is one contiguous 1 MiB read.
    xv = x.rearrange("(i r) (c j) -> i (r c) j", r=R, c=P // R)
    # Store-side view: partition p = 32h + g (g < 16 valid), free (s j).
    ov = out.rearrange("h g s j -> (h g) (s j)")

    with tile.TileContext(nc) as tc:
        with (
            tc.tile_pool(name="const", bufs=1) as cpool,
            tc.tile_pool(name="xp", bufs=12) as xpool,
            tc.tile_pool(name="bp", bufs=6) as bpool,
            tc.tile_pool(name="op", bufs=2) as opool,
            tc.tile_pool(name="ps", bufs=2, space="PSUM") as pspool,
        ):
            # Constants lead the ACT ring; the two load rings are pure
            # x-loads from instruction 0.
            med_t = cpool.tile([P, CQ], mybir.dt.float32)
            nc.scalar.dma_start(out=med_t[:], in_=med[:])
            # Pack weights, cast to bf16 for the PE (values 2^k, exact).
            pw_f32 = cpool.tile([P, G], mybir.dt.float32)
            pw_t = cpool.tile([P, G], mybir.dt.bfloat16)
            nc.scalar.dma_start(out=pw_f32[:], in_=pw[:])
            nc.vector.tensor_copy(out=pw_t[:], in_=pw_f32[:])

            def load(i, c0, w, ring_sel):
                xt = xpool.tile([P, w], mybir.dt.float32, tag="x")
                # Single pure-load queue: splitting loads across two
                # descriptor queues measured ~25% SLOWER — the SDMA
                # engines round-robin packets between queues, so HBM
                # sees two interleaved address streams and loses
                # stream locality.
                nc.sync.dma_start(out=xt[:], in_=xv[i][:, c0 : c0 + w])
                return xt

            def compare(xt, c0, w):
                bt = bpool.tile([P, w], mybir.dt.bfloat16, tag="b")
                nc.vector.tensor_tensor(
                    bt[:], xt[:], med_t[:, c0 : c0 + w], mybir.AluOpType.is_ge
                )
                return bt

            ot, ob = None, 0
            for s in range(N_SUPER):
                for g0, g1 in ST_GROUPS:
                    if s == g0:
                        ot = opool.tile([64, (g1 - g0) * CQ], mybir.dt.uint8, tag="o")
                        ob = g0  # superchunk base of this accumulation tile
                ps = pspool.tile([64, CQ], mybir.dt.float32, tag="ps")
                for ch in (0, 1):
                    i = 2 * s + ch
                    pb = 32 * ch  # PSUM partition base (0 or 32)
                    if 0 < i < N_CHUNKS - 1:
                        xt = load(i, 0, CQ, i)
                        bt = compare(xt, 0, CQ)
                        for b in range(4):
                            nc.tensor.matmul(
                                ps[pb : pb + G, b * MM_N : (b + 1) * MM_N],
                                pw_t[:],
                                bt[:, b * MM_N : (b + 1) * MM_N],
                                start=True,
                                stop=True,
                            )
                    else:
                        # First chunk: 512-col pieces so the DVE
                        # starts ~2.5us earlier. Last chunk: pieces
                        # with the evacuation tapered per col-block so
                        # the post-load drain is one small piece chain.
                        for b in range(4):
                            xt = load(i, b * MM_N, MM_N, b)
                            bt = compare(xt, b * MM_N, MM_N)
                            nc.tensor.matmul(
                                ps[pb : pb + G, b * MM_N : (b + 1) * MM_N],
                                pw_t[:],
                                bt[:],
                                start=True,
                                stop=True,
                            )
                            if i == N_CHUNKS - 1:
                                nc.scalar.copy(
                                    out=ot[
                                        :,
                                        (s - ob) * CQ + b * MM_N :
                                        (s - ob) * CQ + (b + 1) * MM_N,
                                    ],
                                    in_=ps[:, b * MM_N : (b + 1) * MM_N],
                                )
                if s < N_SUPER - 1:
                    # One 64-partition evacuation per chunk pair.
                    nc.scalar.copy(
                        out=ot[:, (s - ob) * CQ : (s - ob + 1) * CQ], in_=ps[:]
                    )
                for g0, g1 in ST_GROUPS:
                    if s == g1 - 1:
                        # Stores ride the ACT ring right behind the
                        # copy they depend on; load queues stay pure.
                        # One plain-AP store per half skips the
                        # garbage rows 16-31/48-63 (composite
                        # partition APs on the SBUF side of a store
                        # scramble partitions — measured, not a
                        # theory).
                        for h in (0, 1):
                            nc.scalar.dma_start(
                                out=ov[
                                    h * G : (h + 1) * G, g0 * CQ : g1 * CQ
                                ],
                                in_=ot[32 * h : 32 * h + G, :],
                            )
    nc.compile()
    return nc


def _pack_weights():
    # x-partition p contributes bit 2^(p%8) to output row p//8.
    pw = np.zeros((P, G), dtype=np.float32)
    for p in range(P):
        pw[p, p // 8] = float(1 << (p % 8))
    return pw


def _in_maps(inputs, medians):
    x = np.ascontiguousarray(np.asarray(inputs, dtype=np.float32))
    m = np.asarray(medians, dtype=np.float32)
    m2 = np.where(m > 0, m, np.float32(np.inf)).astype(np.float32)
    # med_t[p, j] = m2[(p%4)*CQ + j], tiled for all 32 row-groups.
    med = np.ascontiguousarray(
        np.broadcast_to(m2.reshape(1, P // R, CQ), (R, P // R, CQ)).reshape(P, CQ)
    )
    pw = _pack_weights()
    return [
        {"x": x[c * BS : (c + 1) * BS], "med": med, "pw": pw}
        for c in range(N_CORES)
    ]


def _decode(packed):
    """[2, 16, N_SUPER, 2048] u8 -> [BS, F] bool for one core.

    Byte (h, g, s, j) holds x-partitions [8g, 8g+8) of chunk 2s+h at
    chunk-col j; x-partition p = (row-in-chunk p//4, quarter p%4).
    """
    a = packed.reshape(2, G, N_SUPER, CQ, 1)
    bits = np.unpackbits(a, axis=4, bitorder="little")  # [h, g, s, j, k]
    # -> [s, h, g, k, j] = [chunk, x-partition, chunk-col]
    return bits.transpose(2, 0, 1, 4, 3).reshape(BS, F)


def kernel(inputs, medians):
    in_maps = _in_maps(inputs, medians)
    last_err = None
    for attempt in range(4):  # transient axon/NRT failures happen; retry
        try:
            nc = _build()
            res = run_bass_kernel_spmd(nc, in_maps, list(range(N_CORES))).results
            break
        except Exception as e:  # noqa: BLE001
            last_err = e
            # A wedged NeuronCore (NRT_EXEC_UNIT_UNRECOVERABLE) often
            # needs a reset + settle time before the next open works.
            os.environ["NEURON_RT_RESET_CORES"] = "1"
            time.sleep(5 * (attempt + 1))
    else:
        raise last_err
    out = np.concatenate([_decode(r["out"]) for r in res], axis=0)
    return out.astype(bool)
